# revision 25
# baseline (speedup 1.0000x reference)
"""Trainium2 Bass kernel for nn_CIFM_63780264345953.

Reference computation (per batch b of 8):
    S      = (Q @ K^T) * scale_param / sqrt(512)        [N, N]
    A      = softmax(S, axis=-1)
    R      = relu(A @ V)                                [N, D]
    C      = relu((V - R) @ W^T)                        [N, D]
    out    = a * R + b * C

Sharding: data-parallel over batch B=8 across the 8 NeuronCores (one
batch per core). Each core runs the full single-batch attention.

Per-core kernel strategy (N=2048, D=512, fp32 I/O, bf16 matmuls):
  - Load Q, K, V, W with fp32->bf16 cast done in-flight by the SWDGE DMA.
  - Build Q^T, K^T ([d, n] layout) via PE transposes (needed because the
    TensorE contracts along the partition dim).
  - Compute S^T tiles [m(part), n(free)] = (K^T tile).T @ Q^T. exp() on
    ScalarE (scale folded in; no max subtraction - scores are ~N(0,1)
    so exp is safe in fp32/bf16 range).
  - Softmax denominator: a ones-column appended to V makes the A@V
    matmul produce rowsum(exp) for free in the same PSUM accumulation.
  - O' = exp(S)^T.T @ [V|1] accumulated over m; R = relu(O' * recip)
    with the reciprocal applied as the per-partition activation scale.
  - X = V - R (bf16), X^T via PE transposes, C = relu-combined with
    a*R on DVE, DMA out.
"""

import math

import numpy as np

B, N_FULL, D_FULL = 8, 2048, 512
P = 128


def _build_bass(N, D, scale, a_val, b_val, reps=1):
    import concourse.tile as tile
    from concourse import bacc, mybir
    from concourse.masks import make_identity
    from contextlib import ExitStack

    f32 = mybir.dt.float32
    bf16 = mybir.dt.bfloat16
    NB = N // P          # seq blocks (16)
    DB = D // P          # feature blocks (4)
    CH = min(512, N)     # free-dim chunk for S^T matmuls
    NCH = N // CH

    nc = bacc.Bacc(None, num_swdge_queues=4)
    q = nc.declare_dram_parameter("q", [N, D], f32, isOutput=False)
    k = nc.declare_dram_parameter("k", [N, D], f32, isOutput=False)
    v = nc.declare_dram_parameter("v", [N, D], f32, isOutput=False)
    w = nc.declare_dram_parameter("w", [D, D], f32, isOutput=False)
    out = nc.declare_dram_parameter("out", [N, D], f32, isOutput=True)

    q3 = q.rearrange("(nb p) d -> p nb d", p=P)
    k3 = k.rearrange("(nb p) d -> p nb d", p=P)
    v3 = v.rearrange("(nb p) d -> p nb d", p=P)
    w3 = w.rearrange("(ob p) d -> p ob d", p=P)
    out3 = out.rearrange("(nb p) d -> p nb d", p=P)

    G = min(4, NB)        # seq blocks per load chunk
    NG = NB // G          # number of load chunks per tensor

    with ExitStack() as ctx:
        tc = ctx.enter_context(tile.TileContext(nc))

        persist = ctx.enter_context(tc.tile_pool(name="persist", bufs=1))
        qt = persist.tile([P, DB, N], bf16, tag="qt")     # Q^T [d, n]
        kt = persist.tile([P, DB, N], bf16, tag="kt")     # K^T [d, m]
        vhat = persist.tile([P, NB, D + 1], bf16, tag="vhat")  # V | ones
        wt = persist.tile([P, DB, D], bf16, tag="wt")     # W^T [d, o]
        exps = persist.tile([P, NB, N], bf16, tag="exps")  # exp(S^T) [m, n]
        ident = persist.tile([P, P], bf16, tag="ident")
        make_identity(nc, ident)
        # touch exp early so the ACT table set loads during the DMA-bound
        # head instead of on phase 2's critical path
        warm = persist.tile([P, 1], mybir.dt.float32, tag="warm")
        nc.vector.memset(warm, 0.0)
        nc.scalar.activation(out=warm, in_=warm,
                             func=mybir.ActivationFunctionType.Exp)

        conv = ctx.enter_context(tc.tile_pool(name="conv", bufs=3))

        # ---------------- Phase 1: load (cast in DMA) + transpose ---------
        with tc.tile_pool(name="psum_tp", bufs=4, space="PSUM") as psum_tp:
            # Q and K: SWDGE casts fp32->bf16 in flight; PE-transpose into
            # [d, n] layout.
            for src3, dstT in ((q3, qt), (k3, kt)):
                for g in range(NG):
                    cv = conv.tile([P, G, D], bf16, tag="conv")
                    nc.gpsimd.dma_start(out=cv, in_=src3[:, g * G:(g + 1) * G, :])
                    for ds in range(DB):
                        tp = psum_tp.tile([P, G * P], bf16, tag="tp")
                        for j in range(G):
                            nc.tensor.transpose(
                                tp[:, j * P:(j + 1) * P],
                                cv[:, j, ds * P:(ds + 1) * P],
                                ident,
                            )
                        nc.vector.tensor_copy(
                            out=dstT[:, ds, g * G * P:(g + 1) * G * P], in_=tp
                        )
            # V: cast-load bf16 straight into vhat[:, :, 0:D].
            for g in range(NG):
                nc.gpsimd.dma_start(
                    out=vhat[:, g * G:(g + 1) * G, 0:D],
                    in_=v3[:, g * G:(g + 1) * G, :],
                )
            nc.vector.memset(vhat[:, :, D:D + 1], 1.0)
            # W: cast-load, transpose into [d, o] layout.
            cvw = conv.tile([P, DB, D], bf16, tag="conv")
            nc.gpsimd.dma_start(out=cvw, in_=w3)
            for ds in range(DB):
                tp = psum_tp.tile([P, DB * P], bf16, tag="tp")
                for ob in range(DB):
                    nc.tensor.transpose(
                        tp[:, ob * P:(ob + 1) * P],
                        cvw[:, ob, ds * P:(ds + 1) * P],
                        ident,
                    )
                nc.vector.tensor_copy(out=wt[:, ds, :], in_=tp)

        # ---------------- Phase 2: S^T = K Q^T, exp ----------------
        for _rep in range(reps):
            _compute_phases(
                nc, tc, mybir, qt, kt, vhat, wt, exps, ident, out3,
                N, D, NB, DB, CH, NCH, scale, a_val, b_val, _rep,
            )

    nc.finalize()
    return nc


def _compute_phases(nc, tc, mybir, qt, kt, vhat, wt, exps, ident, out3,
                    N, D, NB, DB, CH, NCH, scale, a_val, b_val, rep):
    import concourse.tile as tile  # noqa: F401
    P = 128
    f32 = mybir.dt.float32
    bf16 = mybir.dt.bfloat16
    if True:
        with tc.tile_pool(name=f"psum_s{rep}", bufs=2, space="PSUM") as psum_s:
            for m in range(NB):
                st_ps = psum_s.tile([P, N], mybir.dt.float32, tag="st")
                for ds in range(DB):
                    lhsT = kt[:, ds, m * P:(m + 1) * P]
                    for ncn in range(NCH):
                        nc.tensor.matmul(
                            st_ps[:, ncn * CH:(ncn + 1) * CH],
                            lhsT,
                            qt[:, ds, ncn * CH:(ncn + 1) * CH],
                            start=(ds == 0),
                            stop=(ds == DB - 1),
                        )
                nc.scalar.activation(
                    out=exps[:, m, :],
                    in_=st_ps,
                    func=mybir.ActivationFunctionType.Exp,
                    scale=float(scale),
                )

        # ---------------- Phase 3: O' = exp(S^T).T @ [V|1], R, X, C, out --
        with (
            tc.tile_pool(name=f"psum_av{rep}", bufs=2, space="PSUM") as psum_av,
            tc.tile_pool(name=f"psum_xt{rep}", bufs=2, space="PSUM") as psum_xt,
            tc.tile_pool(name=f"psum_c{rep}", bufs=2, space="PSUM") as psum_c,
            tc.tile_pool(name=f"ph3_{rep}", bufs=3) as ph3,
            tc.tile_pool(name=f"ph3b{rep}", bufs=2) as ph3b,
        ):
            HALF = 256
            for n in range(NB):
                av = psum_av.tile([P, 2, 512], mybir.dt.float32, tag="av")
                for m in range(NB):
                    lhsT = exps[:, m, n * P:(n + 1) * P]
                    nc.tensor.matmul(
                        av[:, 0, 0:HALF],
                        lhsT,
                        vhat[:, m, 0:HALF],
                        start=(m == 0),
                        stop=(m == NB - 1),
                    )
                    nc.tensor.matmul(
                        av[:, 1, 0:HALF + 1],
                        lhsT,
                        vhat[:, m, HALF:D + 1],
                        start=(m == 0),
                        stop=(m == NB - 1),
                    )
                recip = ph3b.tile([P, 1], mybir.dt.float32, tag="recip")
                nc.vector.reciprocal(recip, av[:, 1, HALF:HALF + 1])
                r_t = ph3.tile([P, D], mybir.dt.float32, tag="r")
                nc.scalar.activation(
                    out=r_t.rearrange("p (c f) -> p c f", c=2),
                    in_=av[:, :, 0:HALF],
                    func=mybir.ActivationFunctionType.Relu,
                    scale=recip,
                )
                x_t = ph3b.tile([P, D], bf16, tag="x")
                nc.vector.tensor_tensor(
                    out=x_t, in0=vhat[:, n, 0:D], in1=r_t,
                    op=mybir.AluOpType.subtract,
                )
                xt_ps = psum_xt.tile([P, DB, P], bf16, tag="xt_ps")
                for j in range(DB):
                    nc.tensor.transpose(
                        xt_ps[:, j, :], x_t[:, j * P:(j + 1) * P], ident
                    )
                xt_sb = ph3b.tile([P, DB, P], bf16, tag="xt_sb")
                nc.vector.tensor_copy(out=xt_sb, in_=xt_ps)
                c_ps = psum_c.tile([P, D], mybir.dt.float32, tag="c")
                for ds in range(DB):
                    nc.tensor.matmul(
                        c_ps,
                        xt_sb[:, ds, :],
                        wt[:, ds, :],
                        start=(ds == 0),
                        stop=(ds == DB - 1),
                    )
                # cb = relu(C) * b on ScalarE (b >= 0 folds into the relu
                # scale since relu(b*x) = b*relu(x); otherwise DVE two-op)
                cb_t = ph3b.tile([P, D], mybir.dt.float32, tag="cb")
                if b_val >= 0.0:
                    nc.scalar.activation(
                        out=cb_t, in_=c_ps,
                        func=mybir.ActivationFunctionType.Relu,
                        scale=float(b_val),
                    )
                else:
                    nc.vector.tensor_scalar(
                        out=cb_t, in0=c_ps,
                        scalar1=0.0, scalar2=float(b_val),
                        op0=mybir.AluOpType.max, op1=mybir.AluOpType.mult,
                    )
                o_t = ph3.tile([P, D], mybir.dt.float32, tag="o")
                if a_val == 1.0:
                    nc.vector.tensor_tensor(
                        out=o_t, in0=cb_t, in1=r_t, op=mybir.AluOpType.add
                    )
                else:
                    ra_t = ph3b.tile([P, D], mybir.dt.float32, tag="ra")
                    nc.vector.tensor_scalar(
                        out=ra_t, in0=r_t,
                        scalar1=float(a_val), scalar2=None,
                        op0=mybir.AluOpType.mult,
                    )
                    nc.vector.tensor_tensor(
                        out=o_t, in0=cb_t, in1=ra_t, op=mybir.AluOpType.add
                    )
                nc.gpsimd.dma_start(out=out3[:, n, :], in_=o_t)


def kernel(Q, K, V, W, scale_param, a, b):
    import sys
    if "/opt/trn_rl_repo" not in sys.path:
        sys.path.insert(0, "/opt/trn_rl_repo")
    from concourse.bass_utils import run_bass_kernel_spmd

    Q = np.ascontiguousarray(np.asarray(Q, dtype=np.float32))
    K = np.ascontiguousarray(np.asarray(K, dtype=np.float32))
    V = np.ascontiguousarray(np.asarray(V, dtype=np.float32))
    W = np.ascontiguousarray(np.asarray(W, dtype=np.float32))
    scale = float(np.asarray(scale_param).reshape(-1)[0]) / math.sqrt(D_FULL)
    a_val = float(np.asarray(a).reshape(-1)[0])
    b_val = float(np.asarray(b).reshape(-1)[0])

    nc = _build_bass(N_FULL, D_FULL, scale, a_val, b_val)
    in_maps = [
        {"q": Q[i], "k": K[i], "v": V[i], "w": W} for i in range(B)
    ]
    res = run_bass_kernel_spmd(nc, in_maps, list(range(B)))
    global LAST_RUN
    LAST_RUN = res
    out = np.stack([res.results[i]["out"] for i in range(B)])
    return out.astype(np.float32)


LAST_RUN = None


# revision 32
# speedup vs baseline: 1.0131x; 1.0131x over previous
"""Trainium2 Bass kernel for nn_CIFM_63780264345953.

Reference computation (per batch b of 8):
    S      = (Q @ K^T) * scale_param / sqrt(512)        [N, N]
    A      = softmax(S, axis=-1)
    R      = relu(A @ V)                                [N, D]
    C      = relu((V - R) @ W^T)                        [N, D]
    out    = a * R + b * C

Sharding: data-parallel over batch B=8 across the 8 NeuronCores (one
batch per core). Each core runs the full single-batch attention.

Per-core kernel strategy (N=2048, D=512, fp32 I/O, bf16 matmuls):
  - Load Q, K, V, W with fp32->bf16 cast done in-flight by the SWDGE DMA.
  - Build Q^T, K^T ([d, n] layout) via PE transposes (needed because the
    TensorE contracts along the partition dim).
  - Compute S^T tiles [m(part), n(free)] = (K^T tile).T @ Q^T. exp() on
    ScalarE (scale folded in; no max subtraction - scores are ~N(0,1)
    so exp is safe in fp32/bf16 range).
  - Softmax denominator: a ones-column appended to V makes the A@V
    matmul produce rowsum(exp) for free in the same PSUM accumulation.
  - O' = exp(S)^T.T @ [V|1] accumulated over m; R = relu(O' * recip)
    with the reciprocal applied as the per-partition activation scale.
  - X = V - R (bf16), X^T via PE transposes, C = relu-combined with
    a*R on DVE, DMA out.
"""

import math

import numpy as np

B, N_FULL, D_FULL = 8, 2048, 512
P = 128


def _build_bass(N, D, scale, a_val, b_val, reps=1):
    import concourse.tile as tile
    from concourse import bacc, mybir
    from concourse.masks import make_identity
    from contextlib import ExitStack

    f32 = mybir.dt.float32
    bf16 = mybir.dt.bfloat16
    NB = N // P          # seq blocks (16)
    DB = D // P          # feature blocks (4)
    CH = min(512, N)     # free-dim chunk for S^T matmuls
    NCH = N // CH

    nc = bacc.Bacc(None)
    q = nc.declare_dram_parameter("q", [N, D], f32, isOutput=False)
    k = nc.declare_dram_parameter("k", [N, D], f32, isOutput=False)
    v = nc.declare_dram_parameter("v", [N, D], f32, isOutput=False)
    w = nc.declare_dram_parameter("w", [D, D], f32, isOutput=False)
    out = nc.declare_dram_parameter("out", [N, D], f32, isOutput=True)

    q3 = q.rearrange("(nb p) d -> p nb d", p=P)
    k3 = k.rearrange("(nb p) d -> p nb d", p=P)
    v3 = v.rearrange("(nb p) d -> p nb d", p=P)
    w3 = w.rearrange("(ob p) d -> p ob d", p=P)
    out3 = out.rearrange("(nb p) d -> p nb d", p=P)

    G = min(4, NB)        # seq blocks per load chunk
    NG = NB // G          # number of load chunks per tensor

    with ExitStack() as ctx:
        tc = ctx.enter_context(tile.TileContext(nc))

        persist = ctx.enter_context(tc.tile_pool(name="persist", bufs=1))
        qt = persist.tile([P, DB, N], bf16, tag="qt")     # Q^T [d, n]
        kt = persist.tile([P, DB, N], bf16, tag="kt")     # K^T [d, m]
        vhat = persist.tile([P, NB, D + 1], bf16, tag="vhat")  # V | ones
        wt = persist.tile([P, DB, D], bf16, tag="wt")     # W^T [d, o]
        exps = persist.tile([P, NB, N], bf16, tag="exps")  # exp(S^T) [m, n]
        ident = persist.tile([P, P], bf16, tag="ident")
        make_identity(nc, ident)
        # touch exp early so the ACT table set loads during the DMA-bound
        # head instead of on phase 2's critical path
        warm = persist.tile([P, 1], mybir.dt.float32, tag="warm")
        nc.vector.memset(warm, 0.0)
        nc.scalar.activation(out=warm, in_=warm,
                             func=mybir.ActivationFunctionType.Exp)

        conv = ctx.enter_context(tc.tile_pool(name="conv", bufs=4))

        # ---------------- Phase 1: load (cast in DMA) + transpose ---------
        with tc.tile_pool(name="psum_tp", bufs=2, space="PSUM") as psum_tp:
            # Q and K: SWDGE casts fp32->bf16 in flight; PE-transpose into
            # [d, n] layout. Chunk order (q0, k0), q1..q3, k1..k3: the first
            # S^T m-tile only needs chunk 0 of both, so it can start while
            # the rest stream in.
            chunk_order = [(q3, qt, 0), (k3, kt, 0)]
            chunk_order += [(q3, qt, g) for g in range(1, NG)]
            chunk_order += [(k3, kt, g) for g in range(1, NG)]
            for src3, dstT, g in chunk_order:
                cv = conv.tile([P, G, D], bf16, tag="conv")
                nc.gpsimd.dma_start(out=cv, in_=src3[:, g * G:(g + 1) * G, :])
                for ds in range(DB):
                    tp = psum_tp.tile([P, G * P], bf16, tag="tp")
                    for j in range(G):
                        nc.tensor.transpose(
                            tp[:, j * P:(j + 1) * P],
                            cv[:, j, ds * P:(ds + 1) * P],
                            ident,
                        )
                    nc.vector.tensor_copy(
                        out=dstT[:, ds, g * G * P:(g + 1) * G * P], in_=tp
                    )
            # V: cast-load bf16 straight into vhat[:, :, 0:D].
            for g in range(NG):
                nc.gpsimd.dma_start(
                    out=vhat[:, g * G:(g + 1) * G, 0:D],
                    in_=v3[:, g * G:(g + 1) * G, :],
                )
            nc.vector.memset(vhat[:, :, D:D + 1], 1.0)
            # W: cast-load, transpose into [d, o] layout.
            cvw = conv.tile([P, DB, D], bf16, tag="conv")
            nc.gpsimd.dma_start(out=cvw, in_=w3)
            for ds in range(DB):
                tp = psum_tp.tile([P, DB * P], bf16, tag="tp")
                for ob in range(DB):
                    nc.tensor.transpose(
                        tp[:, ob * P:(ob + 1) * P],
                        cvw[:, ob, ds * P:(ds + 1) * P],
                        ident,
                    )
                nc.vector.tensor_copy(out=wt[:, ds, :], in_=tp)

        # ---------------- Phase 2: S^T = K Q^T, exp ----------------
        for _rep in range(reps):
            _compute_phases(
                nc, tc, mybir, qt, kt, vhat, wt, exps, ident, out3,
                N, D, NB, DB, CH, NCH, scale, a_val, b_val, _rep,
            )

    nc.finalize()
    return nc


def _compute_phases(nc, tc, mybir, qt, kt, vhat, wt, exps, ident, out3,
                    N, D, NB, DB, CH, NCH, scale, a_val, b_val, rep):
    import concourse.tile as tile  # noqa: F401
    P = 128
    f32 = mybir.dt.float32
    bf16 = mybir.dt.bfloat16
    if True:
        # S^T in half-tiles [128, N/2]: finer PSUM turnover and the exp on
        # ScalarE pipelines against the next half's matmuls.
        HNCH = max(1, NCH // 2)          # chunks per half
        HW_ = HNCH * CH                  # half width
        with tc.tile_pool(name=f"psum_s{rep}", bufs=4, space="PSUM") as psum_s:
            for m in range(NB):
                for h in range(N // HW_):
                    st_ps = psum_s.tile([P, HW_], mybir.dt.float32, tag="st")
                    for ds in range(DB):
                        lhsT = kt[:, ds, m * P:(m + 1) * P]
                        for ncn in range(HNCH):
                            nc.tensor.matmul(
                                st_ps[:, ncn * CH:(ncn + 1) * CH],
                                lhsT,
                                qt[:, ds, h * HW_ + ncn * CH:
                                   h * HW_ + (ncn + 1) * CH],
                                start=(ds == 0),
                                stop=(ds == DB - 1),
                            )
                    nc.scalar.activation(
                        out=exps[:, m, h * HW_:(h + 1) * HW_],
                        in_=st_ps,
                        func=mybir.ActivationFunctionType.Exp,
                        scale=float(scale),
                    )

        # ---------------- Phase 3: O' = exp(S^T).T @ [V|1], R, X, C, out --
        with (
            tc.tile_pool(name=f"psum_av{rep}", bufs=2, space="PSUM") as psum_av,
            tc.tile_pool(name=f"psum_xt{rep}", bufs=2, space="PSUM") as psum_xt,
            tc.tile_pool(name=f"psum_c{rep}", bufs=2, space="PSUM") as psum_c,
            tc.tile_pool(name=f"ph3_{rep}", bufs=3) as ph3,
            tc.tile_pool(name=f"ph3b{rep}", bufs=2) as ph3b,
        ):
            HALF = 256
            for n in range(NB):
                av = psum_av.tile([P, 2, 512], mybir.dt.float32, tag="av")
                for m in range(NB):
                    lhsT = exps[:, m, n * P:(n + 1) * P]
                    nc.tensor.matmul(
                        av[:, 0, 0:HALF],
                        lhsT,
                        vhat[:, m, 0:HALF],
                        start=(m == 0),
                        stop=(m == NB - 1),
                    )
                    nc.tensor.matmul(
                        av[:, 1, 0:HALF + 1],
                        lhsT,
                        vhat[:, m, HALF:D + 1],
                        start=(m == 0),
                        stop=(m == NB - 1),
                    )
                recip = ph3b.tile([P, 1], mybir.dt.float32, tag="recip")
                nc.vector.reciprocal(recip, av[:, 1, HALF:HALF + 1])
                r_t = ph3.tile([P, D], mybir.dt.float32, tag="r")
                nc.scalar.activation(
                    out=r_t.rearrange("p (c f) -> p c f", c=2),
                    in_=av[:, :, 0:HALF],
                    func=mybir.ActivationFunctionType.Relu,
                    scale=recip,
                )
                x_t = ph3b.tile([P, D], bf16, tag="x")
                nc.vector.tensor_tensor(
                    out=x_t, in0=vhat[:, n, 0:D], in1=r_t,
                    op=mybir.AluOpType.subtract,
                )
                xt_ps = psum_xt.tile([P, DB, P], bf16, tag="xt_ps")
                for j in range(DB):
                    nc.tensor.transpose(
                        xt_ps[:, j, :], x_t[:, j * P:(j + 1) * P], ident
                    )
                xt_sb = ph3b.tile([P, DB, P], bf16, tag="xt_sb")
                nc.vector.tensor_copy(out=xt_sb, in_=xt_ps)
                c_ps = psum_c.tile([P, D], mybir.dt.float32, tag="c")
                for ds in range(DB):
                    nc.tensor.matmul(
                        c_ps,
                        xt_sb[:, ds, :],
                        wt[:, ds, :],
                        start=(ds == 0),
                        stop=(ds == DB - 1),
                    )
                # cb = relu(C) * b on ScalarE (b >= 0 folds into the relu
                # scale since relu(b*x) = b*relu(x); otherwise DVE two-op)
                cb_t = ph3b.tile([P, D], mybir.dt.float32, tag="cb")
                if b_val >= 0.0:
                    nc.scalar.activation(
                        out=cb_t, in_=c_ps,
                        func=mybir.ActivationFunctionType.Relu,
                        scale=float(b_val),
                    )
                else:
                    nc.vector.tensor_scalar(
                        out=cb_t, in0=c_ps,
                        scalar1=0.0, scalar2=float(b_val),
                        op0=mybir.AluOpType.max, op1=mybir.AluOpType.mult,
                    )
                o_t = ph3.tile([P, D], mybir.dt.float32, tag="o")
                if a_val == 1.0:
                    nc.vector.tensor_tensor(
                        out=o_t, in0=cb_t, in1=r_t, op=mybir.AluOpType.add
                    )
                else:
                    ra_t = ph3b.tile([P, D], mybir.dt.float32, tag="ra")
                    nc.vector.tensor_scalar(
                        out=ra_t, in0=r_t,
                        scalar1=float(a_val), scalar2=None,
                        op0=mybir.AluOpType.mult,
                    )
                    nc.vector.tensor_tensor(
                        out=o_t, in0=cb_t, in1=ra_t, op=mybir.AluOpType.add
                    )
                nc.gpsimd.dma_start(out=out3[:, n, :], in_=o_t)


def kernel(Q, K, V, W, scale_param, a, b):
    import sys
    if "/opt/trn_rl_repo" not in sys.path:
        sys.path.insert(0, "/opt/trn_rl_repo")
    from concourse.bass_utils import run_bass_kernel_spmd

    Q = np.ascontiguousarray(np.asarray(Q, dtype=np.float32))
    K = np.ascontiguousarray(np.asarray(K, dtype=np.float32))
    V = np.ascontiguousarray(np.asarray(V, dtype=np.float32))
    W = np.ascontiguousarray(np.asarray(W, dtype=np.float32))
    scale = float(np.asarray(scale_param).reshape(-1)[0]) / math.sqrt(D_FULL)
    a_val = float(np.asarray(a).reshape(-1)[0])
    b_val = float(np.asarray(b).reshape(-1)[0])

    nc = _build_bass(N_FULL, D_FULL, scale, a_val, b_val)
    in_maps = [
        {"q": Q[i], "k": K[i], "v": V[i], "w": W} for i in range(B)
    ]
    res = run_bass_kernel_spmd(nc, in_maps, list(range(B)))
    global LAST_RUN
    LAST_RUN = res
    out = np.stack([res.results[i]["out"] for i in range(B)])
    return out.astype(np.float32)


LAST_RUN = None
